# revision 5
# baseline (speedup 1.0000x reference)
"""Bahdanau-additive attention scorer on 8 TRN2 NeuronCores.

reference:
  wq = context @ Wc.T            (B, CTX, D)
  uh = queries @ Wq.T + bq       (B, QRS, D)
  scores[b,c,q] = sum_h v[h] * tanh(wq[b,c,h] + uh[b,q,h])
  return scores.reshape(B, QRS, CTX)     # flat view of (B, CTX, QRS)

Sharding: over (batch, query): core k handles batch k//4, queries
(k%4)*64 ... +64, with the full 1024-row context of its batch (context is
replicated across the 4 cores of a batch, weights everywhere).

Device data layout: hidden dim h on partitions (2 tiles of 128 = "m" halves).
Per core:
  prologue (PE, fp16 hi/lo split for fp32 accuracy):
    wqT[h, c]  (2 x [128,1024])   uhT[h, q] + bq  (2 x [128,64])
  main loop over 16 groups of 8 units (unit = (m, q)):
    S[:, u*1024:+1024] = wqT_m + uhT_m[:, q]   DVE tensor_scalar_add (2x mode)
    T = tanh(S)  fp16                          ACT, [128, 8192] per instr
    for each 128-col chunk: scoresT += T_chunk.T @ (v_hi|v_lo)
                                               PE, lhsT=T (fast fp16 load), N=2
  epilogue: DVE adds the v_hi/v_lo column pair, DMA -> DRAM.

Output per core: [128, 512] = scores[b, csub*128 + p, q0 + col//8] with
csub = col % 8; host reassembles + final reshape.
"""

import numpy as np

import concourse.bacc as bacc
import concourse.mybir as mybir
import concourse.tile as tile
from concourse.bass_utils import run_bass_kernel_spmd

F32 = mybir.dt.float32
F16 = mybir.dt.float16
TANH = mybir.ActivationFunctionType.Tanh
ADD = mybir.AluOpType.add
SUB = mybir.AluOpType.subtract

B, CTX, QRS, D = 2, 1024, 256, 256
N_CORES = 8
QL = (B * QRS) // N_CORES        # 64 queries per core
UNITS = 2 * QL                   # (m, q) pairs
GS = 8                           # units per S/T tile
NG = UNITS // GS                 # 16 groups
FREE = GS * CTX                  # 8192


def _split_bf16(nc, pool, src, nfree, tag):
    """hi/lo fp16 split of fp32 SBUF AP src ([128, nfree])."""
    hi = pool.tile([128, nfree], F16, tag=f"{tag}h", name=f"{tag}h")
    lo = pool.tile([128, nfree], F16, tag=f"{tag}l", name=f"{tag}l")
    nc.vector.tensor_copy(hi[:], src)
    nc.vector.tensor_tensor(lo[:], src, hi[:], SUB)
    return hi, lo


def _build_nc():
    nc = bacc.Bacc("TRN2", target_bir_lowering=False, debug=False,
                   enable_asserts=False)

    ctxT = nc.dram_tensor("ctxT", [D, CTX], F32, kind="ExternalInput")
    qT = nc.dram_tensor("qT", [D, QL], F32, kind="ExternalInput")
    WcT = nc.dram_tensor("WcT", [D, D], F32, kind="ExternalInput")
    WqT = nc.dram_tensor("WqT", [D, D], F32, kind="ExternalInput")
    bq2 = nc.dram_tensor("bq2", [128, 2], F32, kind="ExternalInput")
    v2 = nc.dram_tensor("v2", [128, 2], F32, kind="ExternalInput")
    out = nc.dram_tensor("out", [128, 8 * QL], F32, kind="ExternalOutput")

    with tile.TileContext(nc) as tc:
        with (
            tc.tile_pool(name="consts", bufs=1) as cp,
            tc.tile_pool(name="sp", bufs=2) as sp,
            tc.tile_pool(name="tp", bufs=2) as tp,
            tc.tile_pool(name="pre_ps", bufs=2, space="PSUM") as ppre,
            tc.tile_pool(name="out_ps", bufs=1, space="PSUM") as pout,
        ):
            # ---------- load inputs ----------
            ctx_f = [cp.tile([128, CTX], F32, tag=f"ctx{k}", name=f"ctx{k}")
                     for k in range(2)]
            q_f = [cp.tile([128, QL], F32, tag=f"qf{k}", name=f"qf{k}")
                   for k in range(2)]
            wc_f = [cp.tile([128, D], F32, tag=f"wc{k}", name=f"wc{k}")
                    for k in range(2)]
            wq_f = [cp.tile([128, D], F32, tag=f"wqw{k}", name=f"wqw{k}")
                    for k in range(2)]
            for k in range(2):
                sl = slice(k * 128, (k + 1) * 128)
                nc.sync.dma_start(ctx_f[k][:], ctxT[sl, :])
                nc.sync.dma_start(q_f[k][:], qT[sl, :])
                nc.sync.dma_start(wc_f[k][:], WcT[sl, :])
                nc.sync.dma_start(wq_f[k][:], WqT[sl, :])
            bq_t = cp.tile([128, 2], F32, tag="bq", name="bq")
            v_t = cp.tile([128, 2], F32, tag="v", name="v")
            nc.sync.dma_start(bq_t[:], bq2[:])
            nc.sync.dma_start(v_t[:], v2[:])

            # ---------- fp16 hi/lo splits ----------
            ctx_s = [_split_bf16(nc, cp, ctx_f[k][:], CTX, f"cs{k}")
                     for k in range(2)]
            q_s = [_split_bf16(nc, cp, q_f[k][:], QL, f"qs{k}")
                   for k in range(2)]
            wc_s = [_split_bf16(nc, cp, wc_f[k][:], D, f"wcs{k}")
                    for k in range(2)]
            wq_s = [_split_bf16(nc, cp, wq_f[k][:], D, f"wqs{k}")
                    for k in range(2)]
            # v interleaved (vh0, vl0, vh1, vl1)
            vs = cp.tile([128, 4], F16, tag="vs", name="vs")
            for m in range(2):
                nc.vector.tensor_copy(vs[:, 2 * m:2 * m + 1], v_t[:, m:m + 1])
                nc.vector.tensor_tensor(vs[:, 2 * m + 1:2 * m + 2],
                                        v_t[:, m:m + 1],
                                        vs[:, 2 * m:2 * m + 1], SUB)

            # ---------- prologue linear layers ----------
            # pairs (x_part, w_part) skipping lo*lo
            PARTS = [(0, 0), (0, 1), (1, 0)]

            uhT = [cp.tile([128, QL], F32, tag=f"uhT{m}", name=f"uhT{m}")
                   for m in range(2)]
            for m in range(2):
                msl = slice(m * 128, (m + 1) * 128)
                ps_uh = ppre.tile([128, QL], F32, tag="psuh", name=f"psuh{m}")
                first = True
                for xp, wp in PARTS:
                    for k in range(2):
                        nc.tensor.matmul(ps_uh[:], lhsT=wq_s[k][wp][:, msl],
                                         rhs=q_s[k][xp][:],
                                         start=first,
                                         stop=(xp, wp) == PARTS[-1] and k == 1)
                        first = False
                nc.vector.tensor_scalar_add(uhT[m][:], ps_uh[:],
                                            bq_t[:, m:m + 1])

            wqT = [cp.tile([128, CTX], F32, tag=f"wqT{m}", name=f"wqT{m}")
                   for m in range(2)]
            for m in range(2):
                msl = slice(m * 128, (m + 1) * 128)
                for n in range(2):
                    nsl = slice(n * 512, (n + 1) * 512)
                    ps_wq = ppre.tile([128, 512], F32, tag="pswq",
                                      name=f"pswq{m}_{n}")
                    first = True
                    for xp, wp in PARTS:
                        for k in range(2):
                            nc.tensor.matmul(
                                ps_wq[:], lhsT=wc_s[k][wp][:, msl],
                                rhs=ctx_s[k][xp][:, nsl],
                                start=first,
                                stop=(xp, wp) == PARTS[-1] and k == 1)
                            first = False
                    nc.vector.tensor_copy(wqT[m][:, nsl], ps_wq[:])

            # ---------- main loop ----------
            # scoresT psum: [128, 2048]; column (q*8+csub)*4 + m*2 + {hi,lo}
            # every matmul is its own closed accumulation group (one open
            # group per psum bank at a time is a HW/sim constraint)
            ps_out = pout.tile([128, 4 * 8 * QL], F32, tag="pso", name="pso")
            for g in range(NG):
                s = sp.tile([128, FREE], F32, tag="s", name=f"s{g}")
                for j in range(GS):
                    u = g * GS + j
                    m, q = u // QL, u % QL
                    nc.vector.tensor_scalar_add(
                        s[:, j * CTX:(j + 1) * CTX], wqT[m][:],
                        uhT[m][:, q:q + 1])
                t = tp.tile([128, FREE], F16, tag="t", name=f"t{g}")
                nc.scalar.activation(t[:], s[:], TANH)
                for j in range(GS):
                    u = g * GS + j
                    m, q = u // QL, u % QL
                    for csub in range(8):
                        col = 4 * (q * 8 + csub) + 2 * m
                        nc.tensor.matmul(
                            ps_out[:, col:col + 2],
                            lhsT=t[:, j * CTX + csub * 128:
                                   j * CTX + (csub + 1) * 128],
                            rhs=vs[:, 2 * m:2 * m + 2],
                            start=True, stop=True)

            # ---------- epilogue: sum the 4 planes (m x hi/lo), store ----------
            # DVE can read at most one PSUM operand per instruction
            stage = cp.tile([128, 8 * QL], F32, tag="stage", name="stage")
            pr = ps_out[:].rearrange("p (a b) -> p a b", b=4)
            planes = [pr[:, :, i:i + 1].squeeze(2) for i in range(4)]
            nc.vector.tensor_copy(stage[:], planes[0])
            for i in range(1, 4):
                nc.vector.scalar_tensor_tensor(stage[:], planes[i], 0.0,
                                               stage[:], ADD, ADD)
            nc.sync.dma_start(out[:, :], stage[:])

    nc.compile()
    return nc


_NC_CACHE = {}


def _get_nc():
    if "nc" not in _NC_CACHE:
        _NC_CACHE["nc"] = _build_nc()
    return _NC_CACHE["nc"]


def _in_maps(context, queries, Wc, Wq, bq, v):
    WcT = np.ascontiguousarray(Wc.T, dtype=np.float32)
    WqT = np.ascontiguousarray(Wq.T, dtype=np.float32)
    bq2 = np.ascontiguousarray(bq.reshape(2, 128).T, dtype=np.float32)
    v2 = np.ascontiguousarray(v.reshape(2, 128).T, dtype=np.float32)
    ctxTs = [np.ascontiguousarray(context[b].T, dtype=np.float32)
             for b in range(B)]
    maps = []
    for k in range(N_CORES):
        b = k // (N_CORES // B)
        q0 = (k % (N_CORES // B)) * QL
        maps.append({
            "ctxT": ctxTs[b],
            "qT": np.ascontiguousarray(queries[b, q0:q0 + QL, :].T,
                                       dtype=np.float32),
            "WcT": WcT, "WqT": WqT, "bq2": bq2, "v2": v2,
        })
    return maps


def run(context, queries, Wc, Wq, bq, v, trace=False, **spmd_kwargs):
    nc = _get_nc()
    maps = _in_maps(np.asarray(context), np.asarray(queries), np.asarray(Wc),
                    np.asarray(Wq), np.asarray(bq), np.asarray(v))
    res = run_bass_kernel_spmd(nc, maps, core_ids=list(range(N_CORES)),
                               trace=trace, **spmd_kwargs)
    scores = np.empty((B, CTX, QRS), dtype=np.float32)
    for k in range(N_CORES):
        b = k // (N_CORES // B)
        q0 = (k % (N_CORES // B)) * QL
        arr = res.results[k]["out"]            # [128, QL*8]
        # arr[p, q*8+csub] = scores[b, csub*128+p, q0+q]
        blk = arr.reshape(128, QL, 8).transpose(2, 0, 1).reshape(CTX, QL)
        scores[b, :, q0:q0 + QL] = blk
    return scores.reshape(B, QRS, CTX), res


def kernel(context, queries, Wc, Wq, bq, v):
    out, _ = run(context, queries, Wc, Wq, bq, v, trace=False)
    return out
